# revision 35
# baseline (speedup 1.0000x reference)
"""Multi-head self-attention (B=4, S=2048, D=1024, H=16) on 8 TRN2 cores.

Sharding (tensor-parallel over heads, data-parallel over batch): core
c = 2*b + g handles batch b and head-group g (heads 8g..8g+7) over the FULL
sequence.  Each core computes Q/K/V projections for only its 512 feature
columns, its 8 heads' attention, and a PARTIAL output projection
attn_local @ Wo[512g:512(g+1), :].  The host gather adds the two partials
per batch (the "all-reduce" of the Wo row-split runs on the host during
unshard), so there are no device collectives and no duplicated K/V work.

Device layout (everything contracts on the partition dim):
  - Q^T [dq, s] via lhsT=W (natural), rhs=xT.  K^T is stored ZERO-PADDED to
    full 128 contraction rows per head (ktp[:, h, :]: head h's 64 rows live
    at partitions 64h..64h+63, the other 64 rows are zero).  This keeps the
    scores matmuls at K=128 so the PE stays in plain 128x128 mode: mixing
    64-row-tiled and 128-row matmuls costs a pipeline drain per mode switch
    and defeats LDWEIGHTS pull-ahead (~+100ns on every matmul around the
    switch, measured), which is worth far more than the idle PE rows.
  - scores^T [sk, sq] = ktp[:, h].T @ qt, exp on ScalarE (fused 1/8 scale,
    bf16 out).  ScalarE is the bottleneck (~275us of exp) and paces the
    kernel; the PE work per iteration (2 scores + 2 attn + 1 interleaved
    projection matmul, all N=512) is matched to it.
  - cq-lagged software pipeline: slot (pair, cq) emits its own scores/exp,
    the previous slot's attention matmuls (exps long done; ex tiles ride an
    18-deep SBUF ring), and a trickle of the NEXT pair's K/Q projection
    k-steps (K first: slot (p,0) scores need all of ktp).  The V projection
    and pair 0's K1..Q3 chunks trickle through the first slot the same way
    (only K0+Q0 run before slot 0, so exp starts ~15us in).
  - attn^T via lhsT=[V|1] (65 cols): row 64 accumulates the softmax
    denominators for free.
  - normalize: batched DVE reciprocal (split into 4 column pieces so no
    single DVE op head-of-line-blocks the proj-chunk bias adds that gate
    the PE's qp psum rotation), bf16 out, then per cq one PE broadcast
    matmul (0/1 "expander" stationary replicates the two denominator rows
    across partitions) + one DVE multiply.  No DRAM bounce.
  - output projection as TWO bf16 half-partials (pairs 0+1, pairs 2+3):
    the first trickles through the last pair's otherwise-idle slots, the
    second runs at the tail ordered so already-normalized chunks overlap
    the final normalize chain; the host sums 4 partials per batch.  The
    psum->SBUF evacs ride an 8-deep staging ring (each slot is reusable
    only after its output DMA completes, ~2us round trip).
  - prologue DMA choreography: pair-0 Wk/Wq lead the gpsimd queue (the
    sync queue doesn't issue its first DMA until ~14us), x^T arrives in
    three seq-waves interleaved with the small const DMAs, and kt
    zero-pad memsets are deferred behind the wave dispatches they'd
    otherwise block on the shared gpsimd sequencer.
"""

import numpy as np
from contextlib import ExitStack

import concourse.bass as bass
import concourse.mybir as mybir
import concourse.tile as tile
from concourse.bass_utils import run_bass_kernel_spmd

F32 = mybir.dt.float32
BF16 = mybir.dt.bfloat16

D = 1024
S = 2048  # full sequence; every core sees all queries
DG = 512  # feature columns per core (8 heads)
NPAIR = 4  # head pairs per core; pair p = local heads (2p, 2p+1)
NCQ = 4  # query chunks of 512
NSKT = 16  # key chunks of 128
NCORES = 8

# ---------------------------------------------------------------------------
# Workaround: this walrus build rejects >1 sem-wait per instruction ("Too
# many sync wait commands").  After the kernel is fully built, hoist excess
# waits from every instruction onto single-wait NOPs inserted just before it
# in the same engine stream (per-engine program order is preserved, so
# blocking on the NOPs first is equivalent).
# ---------------------------------------------------------------------------


def _split_all_multiwaits(nc):
    n = 0
    for fn in nc.m.functions:
        for bb in fn.blocks:
            lst = bb.instructions
            i = 0
            while i < len(lst):
                inst = lst[i]
                si = inst.sync_info
                if si is not None and si.on_wait is not None and len(si.on_wait) > 1:
                    waits = list(si.on_wait)
                    keep = waits[-1:]
                    del si.on_wait[:]
                    si.on_wait.extend(keep)
                    nops = []
                    for w in waits[:-1]:
                        nop = mybir.InstNoOp(name=f"WSPL-{n}", ins=[], outs=[])
                        n += 1
                        nop.engine = inst.engine
                        nop.sync_info = mybir.SyncInfo(on_wait=[w], on_update=[])
                        nops.append(nop)
                    lst[i:i] = nops
                    i += len(nops)
                i += 1
    return n


def _bcast_ap(dram_handle, nparts, offset_elems, n):
    """DRAM AP replicating a [n] vector across nparts partitions."""
    return bass.AP(
        tensor=dram_handle,
        offset=offset_elems,
        ap=[[0, nparts], [1, n]],
    )


def build_kernel():
    nc = bass.Bass()

    # Weights arrive pre-rearranged from the host into the exact SBUF tile
    # layouts so every weight DMA is a contiguous burst (the on-device
    # "(k p) c -> p k c" gather was ~4us per 256KB on the DIRECT2D path).
    xT = nc.declare_dram_parameter("xT", [D, S], BF16, isOutput=False)
    Wq = nc.declare_dram_parameter("Wq", [NPAIR, 128, 8, 128], BF16, isOutput=False)
    Wk = nc.declare_dram_parameter("Wk", [NPAIR, 128, 8, 128], BF16, isOutput=False)
    Wv = nc.declare_dram_parameter("Wv", [128, 8, DG], BF16, isOutput=False)
    Wo = nc.declare_dram_parameter("Wo", [2, 128, NPAIR, 512], BF16, isOutput=False)
    bq = nc.declare_dram_parameter("bq", [128, NPAIR], F32, isOutput=False)
    bk = nc.declare_dram_parameter("bk", [128, NPAIR], F32, isOutput=False)
    bv = nc.declare_dram_parameter("bv", [DG], F32, isOutput=False)
    bo = nc.declare_dram_parameter("bo", [D], F32, isOutput=False)
    expander_d = nc.declare_dram_parameter("expander", [128, 128], BF16, isOutput=False)
    # Two bf16 half-partials (pairs 0+1, pairs 2+3): the first is computed
    # during the last pair's otherwise-idle slots, the second at the tail.
    # The host sums both halves of both cores per batch (+bo) in f32.
    out = nc.declare_dram_parameter("out", [2, S, D], BF16, isOutput=True)

    Exp = mybir.ActivationFunctionType.Exp

    with tile.TileContext(nc) as tc:
        with ExitStack() as ctx:
            const = ctx.enter_context(tc.tile_pool(name="const", bufs=1))
            xpool = ctx.enter_context(tc.tile_pool(name="xres", bufs=1))
            wqk = ctx.enter_context(tc.tile_pool(name="wqk", bufs=2))
            wv_pool = ctx.enter_context(tc.tile_pool(name="wv", bufs=1))
            qk_pool = ctx.enter_context(tc.tile_pool(name="qk", bufs=2))
            vg_pool = ctx.enter_context(tc.tile_pool(name="vg", bufs=1))
            exp_pool = ctx.enter_context(tc.tile_pool(name="expp", bufs=18))
            small = ctx.enter_context(tc.tile_pool(name="small", bufs=2))
            # 8-deep: each evac copy can only reuse a slot after its DMA
            # completes (~2us round trip incl. the 900ns DMA-sem overhead),
            # so a shallow ring stalls the copy->matmul pipeline at the tail
            out_pool = ctx.enter_context(tc.tile_pool(name="outp", bufs=8))
            wo_pool = ctx.enter_context(tc.tile_pool(name="wo", bufs=2))

            spsum = ctx.enter_context(tc.tile_pool(name="sp", bufs=2, space="PSUM"))
            apsum = ctx.enter_context(tc.tile_pool(name="ap", bufs=1, space="PSUM"))
            qpsum = ctx.enter_context(tc.tile_pool(name="qp", bufs=2, space="PSUM"))
            drpool = ctx.enter_context(tc.tile_pool(name="dr", bufs=2, space="DRAM"))

            # Bias/expander tiles; DMAs are emitted after the x^T wave-1
            # dispatches (the DMA rings hold ~8 outstanding dispatches, and
            # pair-0's weights + wave-1 slices must own the ring heads).
            bq_sb = const.tile([128, NPAIR], F32)
            bk_sb = const.tile([128, NPAIR], F32)
            bv_bc = const.tile([128, DG], F32)
            # 0/1 block pattern: bc = expander.T @ [2,512] replicates moving
            # row 0 onto output partitions 0..63 and row 1 onto 64..127
            # (host-provided: engine memsets can't address partition base 1)
            expander = const.tile([128, 128], BF16)

            def load_consts(which):
                if which == 0:
                    nc.scalar.dma_start(out=bk_sb[:], in_=bk[:, :])
                    nc.scalar.dma_start(out=bq_sb[:], in_=bq[:, :])
                else:
                    # sync queue: not needed before ~+20us, and keeping them
                    # off the scalar sequencer lets the exp-table-preload
                    # activation dispatch by ~+15us instead of ~+22
                    nc.sync.dma_start(
                        out=bv_bc[:], in_=_bcast_ap(bv, 128, 0, DG)
                    )
                    nc.sync.dma_start(out=expander[:], in_=expander_d[:, :])

            # Residents: x^T [d, s] as 8 partition-tiles split across the
            # scalar + gpsimd DMA queues (the sync queue carries the pair-0
            # Wk/Wq and Wv loads first so projection starts immediately).
            xT_sb = xpool.tile([128, 8, S], BF16)
            xT_r = xT.rearrange("(k p) s -> p k s", p=128)

            def load_xt(s_lo, s_hi):
                dma_engines = [nc.scalar, nc.gpsimd]
                for k in range(8):
                    dma_engines[k % 2].dma_start(
                        out=xT_sb[:, k, s_lo:s_hi], in_=xT_r[:, k, s_lo:s_hi]
                    )

            attnT = xpool.tile([128, NPAIR, S], BF16)

            def make_v_steps():
                steps = []
                state = {}
                for skt in range(NSKT):
                    for k in range(8):
                        def step(skt=skt, k=k):
                            if k == 0:
                                state["ps"] = qpsum.tile(
                                    [128, 512], F32, tag="qp", name="vps"
                                )
                            ps = state["ps"]
                            nc.tensor.matmul(
                                ps[:],
                                xT_sb[:, k, 128 * skt : 128 * (skt + 1)],
                                wv_g[:, k, :],
                                start=(k == 0),
                                stop=(k == 7),
                            )
                            if k == 7:
                                nc.vector.tensor_add(
                                    vg[:, skt, :, 0:64],
                                    ps[:].rearrange("p (h d) -> p h d", h=8),
                                    bv_bc[:].rearrange("p (h d) -> p h d", h=8),
                                )
                        steps.append(step)
                return steps

            def start_pair_proj(p, dma_eng=None):
                """DMA the pair's Wq/Wk slices, allocate per-cq-chunk qt /
                zero-padded ktp tiles (chunked so scores only wait on the
                chunks they read), return the 64 per-k-step emission closures.
                Order [K0, Q0, K1, K2, K3, Q1, Q2, Q3]: slot (p, 0) scores
                sweep all K chunks but only Q chunk 0.  Pair 0 passes the
                gpsimd queue: the sync queue doesn't dispatch its first DMA
                until ~14us in, which stalled K0 ~5us."""
                if dma_eng is None:
                    dma_eng = nc.sync
                wk_p = wqk.tile([128, 8, 128], BF16, tag="wk")
                dma_eng.dma_start(out=wk_p[:], in_=Wk[p])
                wq_p = wqk.tile([128, 8, 128], BF16, tag="wq")
                dma_eng.dma_start(out=wq_p[:], in_=Wq[p])
                qt_cs = [
                    qk_pool.tile([128, 512], BF16, tag=f"qt{c}", name=f"qt{c}")
                    for c in range(NCQ)
                ]
                kt_cs = [
                    qk_pool.tile([128, 2, 512], BF16, tag=f"kt{c}", name=f"kt{c}")
                    for c in range(NCQ)
                ]
                def emit_memsets():
                    # gpsimd, not DVE: keeps the zero-pad fills off the DVE
                    # queue so the K-chunk bias adds (which gate the first
                    # scores of the pair) aren't stuck behind them
                    for c in range(NCQ):
                        nc.gpsimd.memset(kt_cs[c][64:128, 0, :], 0.0)
                        nc.gpsimd.memset(kt_cs[c][0:64, 1, :], 0.0)

                if dma_eng is nc.gpsimd:
                    # pair 0: defer so the memsets (gpsimd engine ops) don't
                    # sit between the weight DMAs and the x^T wave-1
                    # dispatches on the shared gpsimd sequencer stream
                    deferred_memsets.append(emit_memsets)
                else:
                    emit_memsets()
                state = {}

                def kstep(c, k):
                    if k == 0:
                        state["ps"] = qpsum.tile(
                            [128, 512], F32, tag="qp", name="kps"
                        )
                    ps = state["ps"]
                    nc.tensor.matmul(
                        ps[:],
                        wk_p[:, k, :],
                        xT_sb[:, k, 512 * c : 512 * (c + 1)],
                        start=(k == 0),
                        stop=(k == 7),
                    )
                    if k == 7:
                        nc.vector.tensor_scalar_add(
                            kt_cs[c][0:64, 0, :],
                            ps[0:64, :],
                            bk_sb[0:64, p : p + 1],
                        )
                        nc.vector.tensor_scalar_add(
                            kt_cs[c][64:128, 1, :],
                            ps[64:128, :],
                            bk_sb[64:128, p : p + 1],
                        )

                def qstep(c, k):
                    if k == 0:
                        state["ps"] = qpsum.tile(
                            [128, 512], F32, tag="qp", name="qps"
                        )
                    ps = state["ps"]
                    nc.tensor.matmul(
                        ps[:],
                        wq_p[:, k, :],
                        xT_sb[:, k, 512 * c : 512 * (c + 1)],
                        start=(k == 0),
                        stop=(k == 7),
                    )
                    if k == 7:
                        nc.vector.tensor_scalar_add(
                            qt_cs[c][:, :], ps[:], bq_sb[:, p : p + 1]
                        )

                chunk_order = [("k", 0), ("q", 0), ("k", 1), ("k", 2), ("k", 3),
                               ("q", 1), ("q", 2), ("q", 3)]
                steps = []
                for which, c in chunk_order:
                    fn = kstep if which == "k" else qstep
                    for k in range(8):
                        steps.append(
                            (lambda fn=fn, c=c, k=k: fn(c, k))
                        )
                return qt_cs, kt_cs, steps

            ex_ring = {}
            pair_states = {}
            deferred_memsets = []

            def make_attn_steps(p, cq):
                """17 closures: 16 lagged attn matmul pairs + psum evacuation."""
                steps = []
                state = {}
                for skt in range(NSKT):
                    def step(skt=skt):
                        if skt == 0:
                            state["aA"] = apsum.tile([65, 512], F32, tag="aA", name="aA")
                            state["aB"] = apsum.tile([65, 512], F32, tag="aB", name="aB")
                        exs = ex_ring.pop((p, cq, skt))
                        nc.tensor.matmul(
                            state["aA"][:],
                            vg[:, skt, 2 * p, :],
                            exs[:, 0:512],
                            start=(skt == 0),
                            stop=(skt == NSKT - 1),
                        )
                        nc.tensor.matmul(
                            state["aB"][:],
                            vg[:, skt, 2 * p + 1, :],
                            exs[:, 512:1024],
                            start=(skt == 0),
                            stop=(skt == NSKT - 1),
                        )
                    steps.append(step)

                def evac():
                    st = pair_states.setdefault(p, {})
                    if "sums" not in st:
                        st["sums"] = small.tile(
                            [128, 1024], F32, tag="sums", name="sums"
                        )
                        # fill with 1.0: the batched reciprocal covers all
                        # 128 partitions, and 1/garbage on the 124 unused
                        # rows can be inf/nan, which the broadcast matmul's
                        # 0-weight would turn into NaN (0*inf)
                        nc.gpsimd.memset(st["sums"][:], 1.0)
                    sums = st["sums"]
                    for half, key in ((0, "aA"), (1, "aB")):
                        at = state[key]
                        # chunk (cq, half) parks at 32-aligned partition
                        # 32*(2*(cq%2)+half), column block 512*(cq//2)
                        nc.vector.tensor_copy(
                            sums[
                                32 * (2 * (cq % 2) + half) : 32
                                * (2 * (cq % 2) + half)
                                + 1,
                                512 * (cq // 2) : 512 * (cq // 2 + 1),
                            ],
                            at[64:65, :],
                        )
                        nc.vector.tensor_copy(
                            attnT[
                                64 * half : 64 * (half + 1),
                                p,
                                512 * cq : 512 * (cq + 1),
                            ],
                            at[0:64, :],
                        )
                steps.append(evac)
                return steps

            def make_norm_steps(p, g):
                """Normalize pair p's cq chunks {2g, 2g+1}: a DVE reciprocal
                (split into 4 column pieces so no single DVE op blocks the
                queue longer than ~1us -- the proj-chunk bias adds that gate
                the PE's qp psum rotation share that queue), then per cq a PE
                broadcast matmul (expander replicates reciprocal rows 0/32
                across partitions 0-63/64-127) and one full-width DVE
                multiply.  Returned as closures so the slot loop can emit
                them interleaved with the skt stream."""
                state = {}

                def recip_piece(i):
                    if "rr" not in state:
                        state["rr"] = small.tile(
                            [128, 512], BF16, tag="rr", name="rr"
                        )
                    sums = pair_states[p]["sums"]
                    # bf16 reciprocal output: keeps the broadcast matmul in
                    # 1-cycle/row bf16 mode (fp32 matmuls are 4 cyc/row);
                    # ~0.1% rounding on the denominator is well in budget
                    with nc.allow_low_precision("bf16 softmax denominators"):
                        nc.vector.reciprocal(
                            state["rr"][:, 128 * i : 128 * (i + 1)],
                            sums[
                                :, 512 * g + 128 * i : 512 * g + 128 * (i + 1)
                            ],
                        )

                def bc_mul(cq):
                    rr = state["rr"]
                    bc = qpsum.tile([128, 512], F32, tag="qp", name="bc")
                    nc.tensor.matmul(
                        bc[:],
                        expander[64 * (cq % 2) : 64 * (cq % 2) + 64, :],
                        rr[64 * (cq % 2) : 64 * (cq % 2) + 64, :],
                        start=True,
                        stop=True,
                    )
                    sl = attnT[:, p, 512 * cq : 512 * (cq + 1)]
                    nc.vector.tensor_mul(sl, sl, bc[:])

                return [lambda i=i: recip_piece(i) for i in range(4)] + [
                    lambda cq=cq: bc_mul(cq) for cq in (2 * g, 2 * g + 1)
                ]

            def normalize_half(p, g):
                for s in make_norm_steps(p, g):
                    s()

            # --- prologue: pair-0 Wk/Wq lead the sync queue (then Wv), x^T
            # on the scalar/gpsimd queues in two waves (seq 0:512 first, so
            # K0/Q0 and scores can start ~40us earlier than a full-x^T
            # wait).  Only K0+Q0 run before slot 0; K1..Q3 and ALL V-chunk
            # projection steps trickle through slot 0 as extras, keeping the
            # PE saturated while the exp stream starts immediately. ---
            # Pair-0 weights lead the gpsimd queue, then x^T wave 1 on the
            # scalar/gpsimd queues; pair-0's kt memsets (gpsimd engine) come
            # after those dispatches so they don't block the queue.
            qt_cur, kt_cur, p0_steps = start_pair_proj(0, dma_eng=nc.gpsimd)
            load_xt(0, 512)
            load_consts(0)
            # seq 512:1024 ahead of the bulky const DMAs: slot-0's V-chunk
            # extras (chunks 4-7) read it by ~skt 6 and stall the PE if the
            # bv broadcast/expander sit ahead of it on the scalar queue
            load_xt(512, 1024)
            load_consts(1)
            for m in deferred_memsets:
                m()
            wv_g = wv_pool.tile([128, 8, DG], BF16)
            nc.sync.dma_start(out=wv_g[:], in_=Wv[:, :, :])
            vg = vg_pool.tile([128, NSKT, 8, 65], BF16)
            # ones column (index 64) via DVE memset: keeps the first EXP's
            # ACT stream free of any DMA-gated instruction (an ACT-side init
            # would serialize table-load + init + first-exp behind the bv
            # broadcast and stall the scores psum rotation ~6us)
            nc.vector.memset(vg[:, :, :, 64:65], 1.0)
            # dummy activation: pulls the 1.28us exp table load to the very
            # start instead of in front of the first real EXP
            scratch = const.tile([1, 4], F32)
            nc.scalar.activation(
                scratch[0:1, 0:4],
                vg[0:1, 0, 0:4, 64],
                Exp,
            )
            # PE warm-up: ~20 junk matmuls on whatever attnT holds, issued
            # while x^T streams in.  Sustained PE activity flips the HAM
            # clock gate to 8/8 (~3.4us of busy-ness) so the real prologue
            # matmuls run at 2.4GHz instead of 1.2 (saves ~15us of cold-rate
            # prologue; results land in a scratch psum tile, never read).
            for _ in range(20):
                jp = qpsum.tile([128, 512], F32, tag="qp", name="jp")
                nc.tensor.matmul(
                    jp[:], attnT[:, 0, 0:128], attnT[:, 0, 1024:1536],
                    start=True, stop=True,
                )
            # K0 + Q0 only (16 steps); the rest rides slot 0's extras.
            for s in p0_steps[:16]:
                s()
            load_xt(1024, S)
            v_steps = make_v_steps()
            v_steps = p0_steps[16:] + v_steps

            # --- main slot stream ---
            slots = [(p, cq) for p in range(NPAIR) for cq in range(NCQ)]
            attn_prev = None
            qt_next = kt_next = None
            proj_next = []

            wo_tiles = []
            norm_pending = []
            OC = 512

            def make_out_steps(half):
                """Half-output-projection steps: partial over pairs (2h, 2h+1)
                into out[half] as bf16.  half 0 trickles through the last
                pair's slots (its attnT is normalized by then and the PE has
                no proj work left); half 1 runs at the tail, ordered so the
                already-normalized q-chunks (t<8, i.e. cq 0/1) run while the
                final normalize chain (reciprocal+bounce+mul) completes."""
                steps = []
                order = [(c, t) for t in range(S // 128) for c in range(D // OC)]
                if half == 1:
                    order = [ct for ct in order if ct[1] < 8] + [
                        ct for ct in order if ct[1] >= 8
                    ]
                for si_, (c, t) in enumerate(order):
                    def step(c=c, t=t, half=half, si_=si_):
                        # tail: deepen the psum rotation to 4 by borrowing
                        # the scores pool's slots (idle once the last exp is
                        # done); reuse the existing tags so no extra PSUM is
                        # allocated
                        if half == 1 and si_ % 2:
                            ps = spsum.tile([128, OC], F32, tag="sc", name="op")
                        else:
                            ps = qpsum.tile([128, OC], F32, tag="qp", name="op")
                        for i in range(2):
                            k = 2 * half + i
                            nc.tensor.matmul(
                                ps[:],
                                attnT[:, k, 128 * t : 128 * (t + 1)],
                                wo_tiles[c][:, k, :],
                                start=(i == 0),
                                stop=(i == 1),
                            )
                        ot = out_pool.tile([128, OC], BF16, tag="ot")
                        # half 0 runs while ACT still paces exp: DVE only.
                        # tail: first 8 steps on ACT (DVE owns the normalize
                        # chain then), after that alternate ACT/DVE.
                        if half == 1 and (si_ < 8 or si_ % 2 == 0):
                            nc.scalar.copy(ot[:], ps[:])
                        else:
                            nc.vector.tensor_copy(ot[:], ps[:])
                        dma_eng = nc.sync if si_ % 2 == 0 else nc.scalar
                        dma_eng.dma_start(
                            out=out[
                                half,
                                128 * t : 128 * (t + 1),
                                OC * c : OC * (c + 1),
                            ],
                            in_=ot[:],
                        )
                    steps.append(step)
                return steps
            for si, (p, cq) in enumerate(slots):
                if cq == 0 and p + 1 < NPAIR:
                    # kick off next pair's weight DMAs + step list one full
                    # pair ahead of first use
                    qt_next, kt_next, proj_next = start_pair_proj(p + 1)
                if si == 11:
                    # prefetch the output projection's Wo chunks
                    for c in range(2):
                        wo_c = wo_pool.tile([128, NPAIR, 512], BF16, tag="wo")
                        nc.sync.dma_start(out=wo_c[:], in_=Wo[c])
                        wo_tiles.append(wo_c)
                if p == NPAIR - 1 and cq == 1:
                    # pairs 0+1 are normalized; their half of the output
                    # projection fills this pair's otherwise-idle PE slots
                    proj_next = make_out_steps(0)
                if si == 0:
                    extras = v_steps
                    v_steps = []
                else:
                    # spread next-pair proj over this pair's remaining slots
                    rem_slots = NCQ - cq
                    take = (
                        (len(proj_next) + rem_slots - 1) // rem_slots
                        if proj_next
                        else 0
                    )
                    extras = proj_next[:take]
                    proj_next = proj_next[take:]
                quota = (len(extras) + NSKT - 1) // NSKT if extras else 0

                ei = 0
                for skt in range(NSKT):
                    if norm_pending and skt % 2 == 0 and skt >= 2:
                        norm_pending.pop(0)()
                    if skt == NSKT - 1 and attn_prev is not None:
                        # early evacuation: previous slot's attn finished at
                        # skt==14 (double-step), so its psum drains during
                        # this iteration and the next slot starts stall-free
                        attn_prev[16]()
                    sc = spsum.tile([128, 1024], F32, tag="sc")
                    ktc = kt_cur[skt // 4]
                    lo = 128 * (skt % 4)
                    nc.tensor.matmul(
                        sc[:, 0:512],
                        ktc[:, 0, lo : lo + 128],
                        qt_cur[cq][:, :],
                        start=True,
                        stop=True,
                    )
                    nc.tensor.matmul(
                        sc[:, 512:1024],
                        ktc[:, 1, lo : lo + 128],
                        qt_cur[cq][:, :],
                        start=True,
                        stop=True,
                    )
                    ex = exp_pool.tile([128, 1024], BF16, tag="ex")
                    nc.scalar.activation(ex[:], sc[:], Exp, scale=0.125)
                    ex_ring[(p, cq, skt)] = ex

                    for _ in range(quota):
                        if ei < len(extras):
                            extras[ei]()
                            ei += 1
                    if attn_prev is not None:
                        if skt < NSKT - 2:
                            attn_prev[skt]()
                        elif skt == NSKT - 2:
                            attn_prev[NSKT - 2]()
                            attn_prev[NSKT - 1]()
                while ei < len(extras):
                    extras[ei]()
                    ei += 1
                while norm_pending:
                    norm_pending.pop(0)()
                if attn_prev is not None:
                    if cq == 0 and p > 0:
                        norm_pending = make_norm_steps(p - 1, 1)
                    elif cq == 2:
                        norm_pending = make_norm_steps(p, 0)
                attn_prev = make_attn_steps(p, cq)
                if cq == NCQ - 1 and p + 1 < NPAIR:
                    qt_cur, kt_cur = qt_next, kt_next

            # --- tail: last slot's attention, then the pairs-2+3 half of
            # the output projection.  The t<8 steps only touch cq 0/1
            # (already normalized), so they keep the PE busy while the last
            # normalize chain (evac -> reciprocal -> bounce -> mul) runs on
            # DVE; the t>=8 steps follow it. ---
            outb = make_out_steps(1)
            for skt in range(NSKT):
                attn_prev[skt]()
            attn_prev[16]()
            for s in outb[:4]:
                s()
            normalize_half(NPAIR - 1, 1)
            for s in outb[4:]:
                s()

    _split_all_multiwaits(nc)
    return nc


_NC_CACHE = None


def _get_nc():
    global _NC_CACHE
    if _NC_CACHE is None:
        _NC_CACHE = build_kernel()
    return _NC_CACHE


def make_in_maps(x, Wq, bq, Wk, bk, Wv, bv, Wo, bo):
    import ml_dtypes

    bf16 = ml_dtypes.bfloat16
    x = np.asarray(x, dtype=np.float32)
    Wq = np.asarray(Wq, dtype=np.float32)
    Wk = np.asarray(Wk, dtype=np.float32)
    Wv = np.asarray(Wv, dtype=np.float32)
    Wo = np.asarray(Wo, dtype=np.float32)
    bq = np.asarray(bq, dtype=np.float32)
    bk = np.asarray(bk, dtype=np.float32)
    bv = np.asarray(bv, dtype=np.float32)
    bo = np.asarray(bo, dtype=np.float32)
    bo_zero = np.zeros_like(bo)

    def qk_layout(w):
        # [D, DG] -> [pair, 128, k, 128]: d = 128k + p_row, col = 128*pair + cc
        return np.ascontiguousarray(
            w.reshape(8, 128, NPAIR, 128).transpose(2, 1, 0, 3).astype(bf16)
        )

    def v_layout(w):
        # [D, DG] -> [128, k, DG]
        return np.ascontiguousarray(
            w.reshape(8, 128, DG).transpose(1, 0, 2).astype(bf16)
        )

    def o_layout(w):
        # [DG, D] -> [chunk, 128, k, 512]: row = 128k + p_row, col = 512c + cc
        return np.ascontiguousarray(
            w.reshape(NPAIR, 128, 2, 512).transpose(2, 1, 0, 3).astype(bf16)
        )

    def b_layout(v):
        # [DG] -> [128, NPAIR]: elem 128k + p_row -> [p_row, k]
        return np.ascontiguousarray(v.reshape(NPAIR, 128).T)

    expander_host = np.zeros((128, 128), dtype=bf16)
    for blk in (0, 64):
        expander_host[blk + 0, 0:64] = 1
        expander_host[blk + 32, 64:128] = 1
    in_maps = []
    for c in range(NCORES):
        b, g = divmod(c, 2)
        lo, hi = DG * g, DG * (g + 1)
        xt = np.ascontiguousarray(x[b].T.astype(bf16))  # [D, S]
        in_maps.append(
            {
                "xT": xt,
                "expander": expander_host,
                "Wq": qk_layout(Wq[:, lo:hi]),
                "Wk": qk_layout(Wk[:, lo:hi]),
                "Wv": v_layout(Wv[:, lo:hi]),
                "Wo": o_layout(Wo[lo:hi, :]),
                "bq": b_layout(bq[lo:hi]),
                "bk": b_layout(bk[lo:hi]),
                "bv": np.ascontiguousarray(bv[lo:hi]),
                "bo": bo if g == 0 else bo_zero,
            }
        )
    return in_maps


def run(x, Wq, bq, Wk, bk, Wv, bv, Wo, bo, trace=False):
    nc = _get_nc()
    in_maps = make_in_maps(x, Wq, bq, Wk, bk, Wv, bv, Wo, bo)
    res = run_bass_kernel_spmd(
        nc, in_maps, core_ids=list(range(NCORES)), trace=trace
    )
    B = 4
    bo = np.asarray(bo, dtype=np.float32)
    out = np.empty((B, S, D), dtype=np.float32)
    for b in range(B):
        # the Wo row-split all-reduce + bias, host-side during unshard;
        # each core ships two bf16 half-partials (pairs 0+1, pairs 2+3)
        pa = np.asarray(res.results[2 * b]["out"], dtype=np.float32)
        pb = np.asarray(res.results[2 * b + 1]["out"], dtype=np.float32)
        out[b] = pa[0] + pa[1] + pb[0] + pb[1]
        out[b] += bo
    return out, res


def kernel(**inputs):
    out, _ = run(**inputs)
    return out



# revision 36
# speedup vs baseline: 1.0104x; 1.0104x over previous
"""Multi-head self-attention (B=4, S=2048, D=1024, H=16) on 8 TRN2 cores.

Sharding (tensor-parallel over heads, data-parallel over batch): core
c = 2*b + g handles batch b and head-group g (heads 8g..8g+7) over the FULL
sequence.  Each core computes Q/K/V projections for only its 512 feature
columns, its 8 heads' attention, and a PARTIAL output projection
attn_local @ Wo[512g:512(g+1), :].  The host gather adds the two partials
per batch (the "all-reduce" of the Wo row-split runs on the host during
unshard), so there are no device collectives and no duplicated K/V work.

Device layout (everything contracts on the partition dim):
  - Q^T [dq, s] via lhsT=W (natural), rhs=xT.  K^T is stored ZERO-PADDED to
    full 128 contraction rows per head (ktp[:, h, :]: head h's 64 rows live
    at partitions 64h..64h+63, the other 64 rows are zero).  This keeps the
    scores matmuls at K=128 so the PE stays in plain 128x128 mode: mixing
    64-row-tiled and 128-row matmuls costs a pipeline drain per mode switch
    and defeats LDWEIGHTS pull-ahead (~+100ns on every matmul around the
    switch, measured), which is worth far more than the idle PE rows.
  - scores^T [sk, sq] = ktp[:, h].T @ qt, exp on ScalarE (fused 1/8 scale,
    bf16 out).  ScalarE is the bottleneck (~275us of exp) and paces the
    kernel; the PE work per iteration (2 scores + 2 attn + 1 interleaved
    projection matmul, all N=512) is matched to it.
  - cq-lagged software pipeline: slot (pair, cq) emits its own scores/exp,
    the previous slot's attention matmuls (exps long done; ex tiles ride an
    18-deep SBUF ring), and a trickle of the NEXT pair's K/Q projection
    k-steps (K first: slot (p,0) scores need all of ktp).  The V projection
    and pair 0's K1..Q3 chunks trickle through the first slot the same way
    (only K0+Q0 run before slot 0, so exp starts ~15us in).
  - attn^T via lhsT=[V|1] (65 cols): row 64 accumulates the softmax
    denominators for free.
  - normalize: batched DVE reciprocal (split into 4 column pieces so no
    single DVE op head-of-line-blocks the proj-chunk bias adds that gate
    the PE's qp psum rotation), bf16 out, then per cq one PE broadcast
    matmul (0/1 "expander" stationary replicates the two denominator rows
    across partitions) + one DVE multiply.  No DRAM bounce.
  - output projection as TWO bf16 half-partials (pairs 0+1, pairs 2+3):
    the first trickles through the last pair's otherwise-idle slots, the
    second runs at the tail ordered so already-normalized chunks overlap
    the final normalize chain; the host sums 4 partials per batch.  The
    psum->SBUF evacs ride an 8-deep staging ring (each slot is reusable
    only after its output DMA completes, ~2us round trip).
  - prologue DMA choreography: pair-0 Wk/Wq lead the gpsimd queue (the
    sync queue doesn't issue its first DMA until ~14us), x^T arrives in
    three seq-waves interleaved with the small const DMAs, and kt
    zero-pad memsets are deferred behind the wave dispatches they'd
    otherwise block on the shared gpsimd sequencer.
"""

import numpy as np
from contextlib import ExitStack

import concourse.bass as bass
import concourse.mybir as mybir
import concourse.tile as tile
from concourse.bass_utils import run_bass_kernel_spmd

F32 = mybir.dt.float32
BF16 = mybir.dt.bfloat16

D = 1024
S = 2048  # full sequence; every core sees all queries
DG = 512  # feature columns per core (8 heads)
NPAIR = 4  # head pairs per core; pair p = local heads (2p, 2p+1)
NCQ = 4  # query chunks of 512
NSKT = 16  # key chunks of 128
NCORES = 8

# ---------------------------------------------------------------------------
# Workaround: this walrus build rejects >1 sem-wait per instruction ("Too
# many sync wait commands").  After the kernel is fully built, hoist excess
# waits from every instruction onto single-wait NOPs inserted just before it
# in the same engine stream (per-engine program order is preserved, so
# blocking on the NOPs first is equivalent).
# ---------------------------------------------------------------------------


def _split_all_multiwaits(nc):
    n = 0
    for fn in nc.m.functions:
        for bb in fn.blocks:
            lst = bb.instructions
            i = 0
            while i < len(lst):
                inst = lst[i]
                si = inst.sync_info
                if si is not None and si.on_wait is not None and len(si.on_wait) > 1:
                    waits = list(si.on_wait)
                    keep = waits[-1:]
                    del si.on_wait[:]
                    si.on_wait.extend(keep)
                    nops = []
                    for w in waits[:-1]:
                        nop = mybir.InstNoOp(name=f"WSPL-{n}", ins=[], outs=[])
                        n += 1
                        nop.engine = inst.engine
                        nop.sync_info = mybir.SyncInfo(on_wait=[w], on_update=[])
                        nops.append(nop)
                    lst[i:i] = nops
                    i += len(nops)
                i += 1
    return n


def _bcast_ap(dram_handle, nparts, offset_elems, n):
    """DRAM AP replicating a [n] vector across nparts partitions."""
    return bass.AP(
        tensor=dram_handle,
        offset=offset_elems,
        ap=[[0, nparts], [1, n]],
    )


def build_kernel():
    nc = bass.Bass()

    # Weights arrive pre-rearranged from the host into the exact SBUF tile
    # layouts so every weight DMA is a contiguous burst (the on-device
    # "(k p) c -> p k c" gather was ~4us per 256KB on the DIRECT2D path).
    xT = nc.declare_dram_parameter("xT", [D, S], BF16, isOutput=False)
    Wq = nc.declare_dram_parameter("Wq", [NPAIR, 128, 8, 128], BF16, isOutput=False)
    Wk = nc.declare_dram_parameter("Wk", [NPAIR, 128, 8, 128], BF16, isOutput=False)
    Wv = nc.declare_dram_parameter("Wv", [128, 8, DG], BF16, isOutput=False)
    Wo = nc.declare_dram_parameter("Wo", [2, 128, NPAIR, 512], BF16, isOutput=False)
    bq = nc.declare_dram_parameter("bq", [128, NPAIR], F32, isOutput=False)
    bk = nc.declare_dram_parameter("bk", [128, NPAIR], F32, isOutput=False)
    bv = nc.declare_dram_parameter("bv", [DG], F32, isOutput=False)
    bo = nc.declare_dram_parameter("bo", [D], F32, isOutput=False)
    expander_d = nc.declare_dram_parameter("expander", [128, 128], BF16, isOutput=False)
    # Two bf16 half-partials (pairs 0+1, pairs 2+3): the first is computed
    # during the last pair's otherwise-idle slots, the second at the tail.
    # The host sums both halves of both cores per batch (+bo) in f32.
    out = nc.declare_dram_parameter("out", [2, S, D], BF16, isOutput=True)

    Exp = mybir.ActivationFunctionType.Exp

    with tile.TileContext(nc) as tc:
        with ExitStack() as ctx:
            const = ctx.enter_context(tc.tile_pool(name="const", bufs=1))
            xpool = ctx.enter_context(tc.tile_pool(name="xres", bufs=1))
            wqk = ctx.enter_context(tc.tile_pool(name="wqk", bufs=2))
            wv_pool = ctx.enter_context(tc.tile_pool(name="wv", bufs=1))
            qk_pool = ctx.enter_context(tc.tile_pool(name="qk", bufs=2))
            vg_pool = ctx.enter_context(tc.tile_pool(name="vg", bufs=1))
            exp_pool = ctx.enter_context(tc.tile_pool(name="expp", bufs=18))
            small = ctx.enter_context(tc.tile_pool(name="small", bufs=2))
            # 8-deep: each evac copy can only reuse a slot after its DMA
            # completes (~2us round trip incl. the 900ns DMA-sem overhead),
            # so a shallow ring stalls the copy->matmul pipeline at the tail
            out_pool = ctx.enter_context(tc.tile_pool(name="outp", bufs=8))
            wo_pool = ctx.enter_context(tc.tile_pool(name="wo", bufs=2))

            spsum = ctx.enter_context(tc.tile_pool(name="sp", bufs=2, space="PSUM"))
            apsum = ctx.enter_context(tc.tile_pool(name="ap", bufs=1, space="PSUM"))
            qpsum = ctx.enter_context(tc.tile_pool(name="qp", bufs=2, space="PSUM"))
            drpool = ctx.enter_context(tc.tile_pool(name="dr", bufs=2, space="DRAM"))

            # Bias/expander tiles; DMAs are emitted after the x^T wave-1
            # dispatches (the DMA rings hold ~8 outstanding dispatches, and
            # pair-0's weights + wave-1 slices must own the ring heads).
            bq_sb = const.tile([128, NPAIR], F32)
            bk_sb = const.tile([128, NPAIR], F32)
            bv_bc = const.tile([128, DG], F32)
            # 0/1 block pattern: bc = expander.T @ [2,512] replicates moving
            # row 0 onto output partitions 0..63 and row 1 onto 64..127
            # (host-provided: engine memsets can't address partition base 1)
            expander = const.tile([128, 128], BF16)

            def load_consts(which):
                if which == 0:
                    nc.scalar.dma_start(out=bk_sb[:], in_=bk[:, :])
                    nc.scalar.dma_start(out=bq_sb[:], in_=bq[:, :])
                else:
                    nc.scalar.dma_start(
                        out=bv_bc[:], in_=_bcast_ap(bv, 128, 0, DG)
                    )
                    nc.scalar.dma_start(out=expander[:], in_=expander_d[:, :])

            # Residents: x^T [d, s] as 8 partition-tiles split across the
            # scalar + gpsimd DMA queues (the sync queue carries the pair-0
            # Wk/Wq and Wv loads first so projection starts immediately).
            xT_sb = xpool.tile([128, 8, S], BF16)
            xT_r = xT.rearrange("(k p) s -> p k s", p=128)

            def load_xt(s_lo, s_hi):
                dma_engines = [nc.scalar, nc.gpsimd]
                for k in range(8):
                    dma_engines[k % 2].dma_start(
                        out=xT_sb[:, k, s_lo:s_hi], in_=xT_r[:, k, s_lo:s_hi]
                    )

            attnT = xpool.tile([128, NPAIR, S], BF16)

            def make_v_steps():
                steps = []
                state = {}
                for skt in range(NSKT):
                    for k in range(8):
                        def step(skt=skt, k=k):
                            if k == 0:
                                state["ps"] = qpsum.tile(
                                    [128, 512], F32, tag="qp", name="vps"
                                )
                            ps = state["ps"]
                            nc.tensor.matmul(
                                ps[:],
                                xT_sb[:, k, 128 * skt : 128 * (skt + 1)],
                                wv_g[:, k, :],
                                start=(k == 0),
                                stop=(k == 7),
                            )
                            if k == 7:
                                nc.vector.tensor_add(
                                    vg[:, skt, :, 0:64],
                                    ps[:].rearrange("p (h d) -> p h d", h=8),
                                    bv_bc[:].rearrange("p (h d) -> p h d", h=8),
                                )
                        steps.append(step)
                return steps

            def start_pair_proj(p, dma_eng=None):
                """DMA the pair's Wq/Wk slices, allocate per-cq-chunk qt /
                zero-padded ktp tiles (chunked so scores only wait on the
                chunks they read), return the 64 per-k-step emission closures.
                Order [K0, Q0, K1, K2, K3, Q1, Q2, Q3]: slot (p, 0) scores
                sweep all K chunks but only Q chunk 0.  Pair 0 passes the
                gpsimd queue: the sync queue doesn't dispatch its first DMA
                until ~14us in, which stalled K0 ~5us."""
                if dma_eng is None:
                    dma_eng = nc.sync
                wk_p = wqk.tile([128, 8, 128], BF16, tag="wk")
                dma_eng.dma_start(out=wk_p[:], in_=Wk[p])
                wq_p = wqk.tile([128, 8, 128], BF16, tag="wq")
                dma_eng.dma_start(out=wq_p[:], in_=Wq[p])
                qt_cs = [
                    qk_pool.tile([128, 512], BF16, tag=f"qt{c}", name=f"qt{c}")
                    for c in range(NCQ)
                ]
                kt_cs = [
                    qk_pool.tile([128, 2, 512], BF16, tag=f"kt{c}", name=f"kt{c}")
                    for c in range(NCQ)
                ]
                def emit_memsets():
                    # gpsimd, not DVE: keeps the zero-pad fills off the DVE
                    # queue so the K-chunk bias adds (which gate the first
                    # scores of the pair) aren't stuck behind them
                    for c in range(NCQ):
                        nc.gpsimd.memset(kt_cs[c][64:128, 0, :], 0.0)
                        nc.gpsimd.memset(kt_cs[c][0:64, 1, :], 0.0)

                if dma_eng is nc.gpsimd:
                    # pair 0: defer so the memsets (gpsimd engine ops) don't
                    # sit between the weight DMAs and the x^T wave-1
                    # dispatches on the shared gpsimd sequencer stream
                    deferred_memsets.append(emit_memsets)
                else:
                    emit_memsets()
                state = {}

                def kstep(c, k):
                    if k == 0:
                        state["ps"] = qpsum.tile(
                            [128, 512], F32, tag="qp", name="kps"
                        )
                    ps = state["ps"]
                    nc.tensor.matmul(
                        ps[:],
                        wk_p[:, k, :],
                        xT_sb[:, k, 512 * c : 512 * (c + 1)],
                        start=(k == 0),
                        stop=(k == 7),
                    )
                    if k == 7:
                        nc.vector.tensor_scalar_add(
                            kt_cs[c][0:64, 0, :],
                            ps[0:64, :],
                            bk_sb[0:64, p : p + 1],
                        )
                        nc.vector.tensor_scalar_add(
                            kt_cs[c][64:128, 1, :],
                            ps[64:128, :],
                            bk_sb[64:128, p : p + 1],
                        )

                def qstep(c, k):
                    if k == 0:
                        state["ps"] = qpsum.tile(
                            [128, 512], F32, tag="qp", name="qps"
                        )
                    ps = state["ps"]
                    nc.tensor.matmul(
                        ps[:],
                        wq_p[:, k, :],
                        xT_sb[:, k, 512 * c : 512 * (c + 1)],
                        start=(k == 0),
                        stop=(k == 7),
                    )
                    if k == 7:
                        nc.vector.tensor_scalar_add(
                            qt_cs[c][:, :], ps[:], bq_sb[:, p : p + 1]
                        )

                chunk_order = [("k", 0), ("q", 0), ("k", 1), ("k", 2), ("k", 3),
                               ("q", 1), ("q", 2), ("q", 3)]
                steps = []
                for which, c in chunk_order:
                    fn = kstep if which == "k" else qstep
                    for k in range(8):
                        steps.append(
                            (lambda fn=fn, c=c, k=k: fn(c, k))
                        )
                return qt_cs, kt_cs, steps

            ex_ring = {}
            pair_states = {}
            deferred_memsets = []

            def make_attn_steps(p, cq):
                """17 closures: 16 lagged attn matmul pairs + psum evacuation."""
                steps = []
                state = {}
                for skt in range(NSKT):
                    def step(skt=skt):
                        if skt == 0:
                            state["aA"] = apsum.tile([65, 512], F32, tag="aA", name="aA")
                            state["aB"] = apsum.tile([65, 512], F32, tag="aB", name="aB")
                        exs = ex_ring.pop((p, cq, skt))
                        nc.tensor.matmul(
                            state["aA"][:],
                            vg[:, skt, 2 * p, :],
                            exs[:, 0:512],
                            start=(skt == 0),
                            stop=(skt == NSKT - 1),
                        )
                        nc.tensor.matmul(
                            state["aB"][:],
                            vg[:, skt, 2 * p + 1, :],
                            exs[:, 512:1024],
                            start=(skt == 0),
                            stop=(skt == NSKT - 1),
                        )
                    steps.append(step)

                def evac():
                    st = pair_states.setdefault(p, {})
                    if "sums" not in st:
                        st["sums"] = small.tile(
                            [128, 1024], F32, tag="sums", name="sums"
                        )
                        # fill with 1.0: the batched reciprocal covers all
                        # 128 partitions, and 1/garbage on the 124 unused
                        # rows can be inf/nan, which the broadcast matmul's
                        # 0-weight would turn into NaN (0*inf)
                        nc.gpsimd.memset(st["sums"][:], 1.0)
                    sums = st["sums"]
                    for half, key in ((0, "aA"), (1, "aB")):
                        at = state[key]
                        # chunk (cq, half) parks at 32-aligned partition
                        # 32*(2*(cq%2)+half), column block 512*(cq//2)
                        nc.vector.tensor_copy(
                            sums[
                                32 * (2 * (cq % 2) + half) : 32
                                * (2 * (cq % 2) + half)
                                + 1,
                                512 * (cq // 2) : 512 * (cq // 2 + 1),
                            ],
                            at[64:65, :],
                        )
                        nc.vector.tensor_copy(
                            attnT[
                                64 * half : 64 * (half + 1),
                                p,
                                512 * cq : 512 * (cq + 1),
                            ],
                            at[0:64, :],
                        )
                steps.append(evac)
                return steps

            def make_norm_steps(p, g):
                """Normalize pair p's cq chunks {2g, 2g+1}: a DVE reciprocal
                (split into 4 column pieces so no single DVE op blocks the
                queue longer than ~1us -- the proj-chunk bias adds that gate
                the PE's qp psum rotation share that queue), then per cq a PE
                broadcast matmul (expander replicates reciprocal rows 0/32
                across partitions 0-63/64-127) and one full-width DVE
                multiply.  Returned as closures so the slot loop can emit
                them interleaved with the skt stream."""
                state = {}

                def recip_piece(i):
                    if "rr" not in state:
                        state["rr"] = small.tile(
                            [128, 512], BF16, tag="rr", name="rr"
                        )
                    sums = pair_states[p]["sums"]
                    # bf16 reciprocal output: keeps the broadcast matmul in
                    # 1-cycle/row bf16 mode (fp32 matmuls are 4 cyc/row);
                    # ~0.1% rounding on the denominator is well in budget
                    with nc.allow_low_precision("bf16 softmax denominators"):
                        nc.vector.reciprocal(
                            state["rr"][:, 128 * i : 128 * (i + 1)],
                            sums[
                                :, 512 * g + 128 * i : 512 * g + 128 * (i + 1)
                            ],
                        )

                def bc_mul(cq):
                    rr = state["rr"]
                    bc = qpsum.tile([128, 512], F32, tag="qp", name="bc")
                    nc.tensor.matmul(
                        bc[:],
                        expander[64 * (cq % 2) : 64 * (cq % 2) + 64, :],
                        rr[64 * (cq % 2) : 64 * (cq % 2) + 64, :],
                        start=True,
                        stop=True,
                    )
                    sl = attnT[:, p, 512 * cq : 512 * (cq + 1)]
                    nc.vector.tensor_mul(sl, sl, bc[:])

                return [lambda i=i: recip_piece(i) for i in range(4)] + [
                    lambda cq=cq: bc_mul(cq) for cq in (2 * g, 2 * g + 1)
                ]

            def normalize_half(p, g):
                for s in make_norm_steps(p, g):
                    s()

            # --- prologue: pair-0 Wk/Wq lead the sync queue (then Wv), x^T
            # on the scalar/gpsimd queues in two waves (seq 0:512 first, so
            # K0/Q0 and scores can start ~40us earlier than a full-x^T
            # wait).  Only K0+Q0 run before slot 0; K1..Q3 and ALL V-chunk
            # projection steps trickle through slot 0 as extras, keeping the
            # PE saturated while the exp stream starts immediately. ---
            # Pair-0 weights lead the gpsimd queue, then x^T wave 1 on the
            # scalar/gpsimd queues; pair-0's kt memsets (gpsimd engine) come
            # after those dispatches so they don't block the queue.
            qt_cur, kt_cur, p0_steps = start_pair_proj(0, dma_eng=nc.gpsimd)
            load_xt(0, 512)
            load_consts(0)
            # seq 512:1024 ahead of the bulky const DMAs: slot-0's V-chunk
            # extras (chunks 4-7) read it by ~skt 6 and stall the PE if the
            # bv broadcast/expander sit ahead of it on the scalar queue
            load_xt(512, 1024)
            load_consts(1)
            for m in deferred_memsets:
                m()
            wv_g = wv_pool.tile([128, 8, DG], BF16)
            nc.sync.dma_start(out=wv_g[:], in_=Wv[:, :, :])
            vg = vg_pool.tile([128, NSKT, 8, 65], BF16)
            # ones column (index 64) via DVE memset: keeps the first EXP's
            # ACT stream free of any DMA-gated instruction (an ACT-side init
            # would serialize table-load + init + first-exp behind the bv
            # broadcast and stall the scores psum rotation ~6us)
            nc.vector.memset(vg[:, :, :, 64:65], 1.0)
            # dummy activation: pulls the 1.28us exp table load to the very
            # start instead of in front of the first real EXP
            scratch = const.tile([1, 4], F32)
            nc.scalar.activation(
                scratch[0:1, 0:4],
                vg[0:1, 0, 0:4, 64],
                Exp,
            )
            # PE warm-up: ~20 junk matmuls on whatever attnT holds, issued
            # while x^T streams in.  Sustained PE activity flips the HAM
            # clock gate to 8/8 (~3.4us of busy-ness) so the real prologue
            # matmuls run at 2.4GHz instead of 1.2 (saves ~15us of cold-rate
            # prologue; results land in a scratch psum tile, never read).
            for _ in range(20):
                jp = qpsum.tile([128, 512], F32, tag="qp", name="jp")
                nc.tensor.matmul(
                    jp[:], attnT[:, 0, 0:128], attnT[:, 0, 1024:1536],
                    start=True, stop=True,
                )
            # K0 + Q0 only (16 steps); the rest rides slot 0's extras.
            for s in p0_steps[:16]:
                s()
            load_xt(1024, S)
            v_steps = make_v_steps()
            v_steps = p0_steps[16:] + v_steps

            # --- main slot stream ---
            slots = [(p, cq) for p in range(NPAIR) for cq in range(NCQ)]
            attn_prev = None
            qt_next = kt_next = None
            proj_next = []

            wo_tiles = []
            norm_pending = []
            OC = 512

            def make_out_steps(half):
                """Half-output-projection steps: partial over pairs (2h, 2h+1)
                into out[half] as bf16.  half 0 trickles through the last
                pair's slots (its attnT is normalized by then and the PE has
                no proj work left); half 1 runs at the tail, ordered so the
                already-normalized q-chunks (t<8, i.e. cq 0/1) run while the
                final normalize chain (reciprocal+bounce+mul) completes."""
                steps = []
                order = [(c, t) for t in range(S // 128) for c in range(D // OC)]
                if half == 1:
                    order = [ct for ct in order if ct[1] < 8] + [
                        ct for ct in order if ct[1] >= 8
                    ]
                for si_, (c, t) in enumerate(order):
                    def step(c=c, t=t, half=half, si_=si_):
                        # tail: deepen the psum rotation to 4 by borrowing
                        # the scores pool's slots (idle once the last exp is
                        # done); reuse the existing tags so no extra PSUM is
                        # allocated
                        if half == 1 and si_ % 2:
                            ps = spsum.tile([128, OC], F32, tag="sc", name="op")
                        else:
                            ps = qpsum.tile([128, OC], F32, tag="qp", name="op")
                        for i in range(2):
                            k = 2 * half + i
                            nc.tensor.matmul(
                                ps[:],
                                attnT[:, k, 128 * t : 128 * (t + 1)],
                                wo_tiles[c][:, k, :],
                                start=(i == 0),
                                stop=(i == 1),
                            )
                        ot = out_pool.tile([128, OC], BF16, tag="ot")
                        # half 0 runs while ACT still paces exp: DVE only.
                        # tail: first 8 steps on ACT (DVE owns the normalize
                        # chain then), after that alternate ACT/DVE.
                        if half == 1 and (si_ < 8 or si_ % 2 == 0):
                            nc.scalar.copy(ot[:], ps[:])
                        else:
                            nc.vector.tensor_copy(ot[:], ps[:])
                        dma_eng = nc.sync if si_ % 2 == 0 else nc.scalar
                        dma_eng.dma_start(
                            out=out[
                                half,
                                128 * t : 128 * (t + 1),
                                OC * c : OC * (c + 1),
                            ],
                            in_=ot[:],
                        )
                    steps.append(step)
                return steps
            for si, (p, cq) in enumerate(slots):
                if cq == 0 and p + 1 < NPAIR:
                    # kick off next pair's weight DMAs + step list one full
                    # pair ahead of first use
                    qt_next, kt_next, proj_next = start_pair_proj(p + 1)
                if si == 11:
                    # prefetch the output projection's Wo chunks
                    for c in range(2):
                        wo_c = wo_pool.tile([128, NPAIR, 512], BF16, tag="wo")
                        nc.sync.dma_start(out=wo_c[:], in_=Wo[c])
                        wo_tiles.append(wo_c)
                if p == NPAIR - 1 and cq == 1:
                    # pairs 0+1 are normalized; their half of the output
                    # projection fills this pair's otherwise-idle PE slots
                    proj_next = make_out_steps(0)
                if si == 0:
                    extras = v_steps
                    v_steps = []
                else:
                    # spread next-pair proj over this pair's remaining slots
                    rem_slots = NCQ - cq
                    take = (
                        (len(proj_next) + rem_slots - 1) // rem_slots
                        if proj_next
                        else 0
                    )
                    extras = proj_next[:take]
                    proj_next = proj_next[take:]
                quota = (len(extras) + NSKT - 1) // NSKT if extras else 0

                ei = 0
                for skt in range(NSKT):
                    if norm_pending and skt % 2 == 0 and skt >= 2:
                        norm_pending.pop(0)()
                    if skt == NSKT - 1 and attn_prev is not None:
                        # early evacuation: previous slot's attn finished at
                        # skt==14 (double-step), so its psum drains during
                        # this iteration and the next slot starts stall-free
                        attn_prev[16]()
                    sc = spsum.tile([128, 1024], F32, tag="sc")
                    ktc = kt_cur[skt // 4]
                    lo = 128 * (skt % 4)
                    nc.tensor.matmul(
                        sc[:, 0:512],
                        ktc[:, 0, lo : lo + 128],
                        qt_cur[cq][:, :],
                        start=True,
                        stop=True,
                    )
                    nc.tensor.matmul(
                        sc[:, 512:1024],
                        ktc[:, 1, lo : lo + 128],
                        qt_cur[cq][:, :],
                        start=True,
                        stop=True,
                    )
                    ex = exp_pool.tile([128, 1024], BF16, tag="ex")
                    nc.scalar.activation(ex[:], sc[:], Exp, scale=0.125)
                    ex_ring[(p, cq, skt)] = ex

                    for _ in range(quota):
                        if ei < len(extras):
                            extras[ei]()
                            ei += 1
                    if attn_prev is not None:
                        if skt < NSKT - 2:
                            attn_prev[skt]()
                        elif skt == NSKT - 2:
                            attn_prev[NSKT - 2]()
                            attn_prev[NSKT - 1]()
                while ei < len(extras):
                    extras[ei]()
                    ei += 1
                while norm_pending:
                    norm_pending.pop(0)()
                if attn_prev is not None:
                    if cq == 0 and p > 0:
                        norm_pending = make_norm_steps(p - 1, 1)
                    elif cq == 2:
                        norm_pending = make_norm_steps(p, 0)
                attn_prev = make_attn_steps(p, cq)
                if cq == NCQ - 1 and p + 1 < NPAIR:
                    qt_cur, kt_cur = qt_next, kt_next

            # --- tail: last slot's attention, then the pairs-2+3 half of
            # the output projection.  The t<8 steps only touch cq 0/1
            # (already normalized), so they keep the PE busy while the last
            # normalize chain (evac -> reciprocal -> bounce -> mul) runs on
            # DVE; the t>=8 steps follow it. ---
            outb = make_out_steps(1)
            for skt in range(NSKT):
                attn_prev[skt]()
            attn_prev[16]()
            for s in outb[:4]:
                s()
            normalize_half(NPAIR - 1, 1)
            for s in outb[4:]:
                s()

    _split_all_multiwaits(nc)
    return nc


_NC_CACHE = None


def _get_nc():
    global _NC_CACHE
    if _NC_CACHE is None:
        _NC_CACHE = build_kernel()
    return _NC_CACHE


def make_in_maps(x, Wq, bq, Wk, bk, Wv, bv, Wo, bo):
    import ml_dtypes

    bf16 = ml_dtypes.bfloat16
    x = np.asarray(x, dtype=np.float32)
    Wq = np.asarray(Wq, dtype=np.float32)
    Wk = np.asarray(Wk, dtype=np.float32)
    Wv = np.asarray(Wv, dtype=np.float32)
    Wo = np.asarray(Wo, dtype=np.float32)
    bq = np.asarray(bq, dtype=np.float32)
    bk = np.asarray(bk, dtype=np.float32)
    bv = np.asarray(bv, dtype=np.float32)
    bo = np.asarray(bo, dtype=np.float32)
    bo_zero = np.zeros_like(bo)

    def qk_layout(w):
        # [D, DG] -> [pair, 128, k, 128]: d = 128k + p_row, col = 128*pair + cc
        return np.ascontiguousarray(
            w.reshape(8, 128, NPAIR, 128).transpose(2, 1, 0, 3).astype(bf16)
        )

    def v_layout(w):
        # [D, DG] -> [128, k, DG]
        return np.ascontiguousarray(
            w.reshape(8, 128, DG).transpose(1, 0, 2).astype(bf16)
        )

    def o_layout(w):
        # [DG, D] -> [chunk, 128, k, 512]: row = 128k + p_row, col = 512c + cc
        return np.ascontiguousarray(
            w.reshape(NPAIR, 128, 2, 512).transpose(2, 1, 0, 3).astype(bf16)
        )

    def b_layout(v):
        # [DG] -> [128, NPAIR]: elem 128k + p_row -> [p_row, k]
        return np.ascontiguousarray(v.reshape(NPAIR, 128).T)

    expander_host = np.zeros((128, 128), dtype=bf16)
    for blk in (0, 64):
        expander_host[blk + 0, 0:64] = 1
        expander_host[blk + 32, 64:128] = 1
    in_maps = []
    for c in range(NCORES):
        b, g = divmod(c, 2)
        lo, hi = DG * g, DG * (g + 1)
        xt = np.ascontiguousarray(x[b].T.astype(bf16))  # [D, S]
        in_maps.append(
            {
                "xT": xt,
                "expander": expander_host,
                "Wq": qk_layout(Wq[:, lo:hi]),
                "Wk": qk_layout(Wk[:, lo:hi]),
                "Wv": v_layout(Wv[:, lo:hi]),
                "Wo": o_layout(Wo[lo:hi, :]),
                "bq": b_layout(bq[lo:hi]),
                "bk": b_layout(bk[lo:hi]),
                "bv": np.ascontiguousarray(bv[lo:hi]),
                "bo": bo if g == 0 else bo_zero,
            }
        )
    return in_maps


def run(x, Wq, bq, Wk, bk, Wv, bv, Wo, bo, trace=False):
    nc = _get_nc()
    in_maps = make_in_maps(x, Wq, bq, Wk, bk, Wv, bv, Wo, bo)
    res = run_bass_kernel_spmd(
        nc, in_maps, core_ids=list(range(NCORES)), trace=trace
    )
    B = 4
    bo = np.asarray(bo, dtype=np.float32)
    out = np.empty((B, S, D), dtype=np.float32)
    for b in range(B):
        # the Wo row-split all-reduce + bias, host-side during unshard;
        # each core ships two bf16 half-partials (pairs 0+1, pairs 2+3)
        pa = np.asarray(res.results[2 * b]["out"], dtype=np.float32)
        pb = np.asarray(res.results[2 * b + 1]["out"], dtype=np.float32)
        out[b] = pa[0] + pa[1] + pb[0] + pb[1]
        out[b] += bo
    return out, res


def kernel(**inputs):
    out, _ = run(**inputs)
    return out



# revision 37
# speedup vs baseline: 1.0168x; 1.0063x over previous
"""Multi-head self-attention (B=4, S=2048, D=1024, H=16) on 8 TRN2 cores.

Sharding (tensor-parallel over heads, data-parallel over batch): core
c = 2*b + g handles batch b and head-group g (heads 8g..8g+7) over the FULL
sequence.  Each core computes Q/K/V projections for only its 512 feature
columns, its 8 heads' attention, and a PARTIAL output projection
attn_local @ Wo[512g:512(g+1), :].  The host gather adds the two partials
per batch (the "all-reduce" of the Wo row-split runs on the host during
unshard), so there are no device collectives and no duplicated K/V work.

Device layout (everything contracts on the partition dim):
  - Q^T [dq, s] via lhsT=W (natural), rhs=xT.  K^T is stored ZERO-PADDED to
    full 128 contraction rows per head (ktp[:, h, :]: head h's 64 rows live
    at partitions 64h..64h+63, the other 64 rows are zero).  This keeps the
    scores matmuls at K=128 so the PE stays in plain 128x128 mode: mixing
    64-row-tiled and 128-row matmuls costs a pipeline drain per mode switch
    and defeats LDWEIGHTS pull-ahead (~+100ns on every matmul around the
    switch, measured), which is worth far more than the idle PE rows.
  - scores^T [sk, sq] = ktp[:, h].T @ qt, exp on ScalarE (fused 1/8 scale,
    bf16 out).  ScalarE is the bottleneck (~275us of exp) and paces the
    kernel; the PE work per iteration (2 scores + 2 attn + 1 interleaved
    projection matmul, all N=512) is matched to it.
  - cq-lagged software pipeline: slot (pair, cq) emits its own scores/exp,
    the previous slot's attention matmuls (exps long done; ex tiles ride an
    18-deep SBUF ring), and a trickle of the NEXT pair's K/Q projection
    k-steps (K first: slot (p,0) scores need all of ktp).  The V projection
    and pair 0's K1..Q3 chunks trickle through the first slot the same way
    (only K0+Q0 run before slot 0, so exp starts ~15us in).
  - attn^T via lhsT=[V|1] (65 cols): row 64 accumulates the softmax
    denominators for free.
  - normalize: batched DVE reciprocal (split into 4 column pieces so no
    single DVE op head-of-line-blocks the proj-chunk bias adds that gate
    the PE's qp psum rotation), bf16 out, then per cq one PE broadcast
    matmul (0/1 "expander" stationary replicates the two denominator rows
    across partitions) + one DVE multiply.  No DRAM bounce.
  - output projection as TWO bf16 half-partials (pairs 0+1, pairs 2+3):
    the first trickles through the last pair's otherwise-idle slots, the
    second runs at the tail ordered so already-normalized chunks overlap
    the final normalize chain; the host sums 4 partials per batch.  The
    psum->SBUF evacs ride an 8-deep staging ring (each slot is reusable
    only after its output DMA completes, ~2us round trip).
  - prologue DMA choreography: pair-0 Wk/Wq lead the gpsimd queue (the
    sync queue doesn't issue its first DMA until ~14us), x^T arrives in
    three seq-waves interleaved with the small const DMAs, and kt
    zero-pad memsets are deferred behind the wave dispatches they'd
    otherwise block on the shared gpsimd sequencer.
"""

import numpy as np
from contextlib import ExitStack

import concourse.bass as bass
import concourse.mybir as mybir
import concourse.tile as tile
from concourse.bass_utils import run_bass_kernel_spmd

F32 = mybir.dt.float32
BF16 = mybir.dt.bfloat16

D = 1024
S = 2048  # full sequence; every core sees all queries
DG = 512  # feature columns per core (8 heads)
NPAIR = 4  # head pairs per core; pair p = local heads (2p, 2p+1)
NCQ = 4  # query chunks of 512
NSKT = 16  # key chunks of 128
NCORES = 8

# ---------------------------------------------------------------------------
# Workaround: this walrus build rejects >1 sem-wait per instruction ("Too
# many sync wait commands").  After the kernel is fully built, hoist excess
# waits from every instruction onto single-wait NOPs inserted just before it
# in the same engine stream (per-engine program order is preserved, so
# blocking on the NOPs first is equivalent).
# ---------------------------------------------------------------------------


def _split_all_multiwaits(nc):
    n = 0
    for fn in nc.m.functions:
        for bb in fn.blocks:
            lst = bb.instructions
            i = 0
            while i < len(lst):
                inst = lst[i]
                si = inst.sync_info
                if si is not None and si.on_wait is not None and len(si.on_wait) > 1:
                    waits = list(si.on_wait)
                    keep = waits[-1:]
                    del si.on_wait[:]
                    si.on_wait.extend(keep)
                    nops = []
                    for w in waits[:-1]:
                        nop = mybir.InstNoOp(name=f"WSPL-{n}", ins=[], outs=[])
                        n += 1
                        nop.engine = inst.engine
                        nop.sync_info = mybir.SyncInfo(on_wait=[w], on_update=[])
                        nops.append(nop)
                    lst[i:i] = nops
                    i += len(nops)
                i += 1
    return n


def _bcast_ap(dram_handle, nparts, offset_elems, n):
    """DRAM AP replicating a [n] vector across nparts partitions."""
    return bass.AP(
        tensor=dram_handle,
        offset=offset_elems,
        ap=[[0, nparts], [1, n]],
    )


def build_kernel():
    nc = bass.Bass()

    # Weights arrive pre-rearranged from the host into the exact SBUF tile
    # layouts so every weight DMA is a contiguous burst (the on-device
    # "(k p) c -> p k c" gather was ~4us per 256KB on the DIRECT2D path).
    xT = nc.declare_dram_parameter("xT", [D, S], BF16, isOutput=False)
    Wq = nc.declare_dram_parameter("Wq", [NPAIR, 128, 8, 128], BF16, isOutput=False)
    Wk = nc.declare_dram_parameter("Wk", [NPAIR, 128, 8, 128], BF16, isOutput=False)
    Wv = nc.declare_dram_parameter("Wv", [128, 8, DG], BF16, isOutput=False)
    Wo = nc.declare_dram_parameter("Wo", [2, 128, NPAIR, 512], BF16, isOutput=False)
    bq = nc.declare_dram_parameter("bq", [128, NPAIR], F32, isOutput=False)
    bk = nc.declare_dram_parameter("bk", [128, NPAIR], F32, isOutput=False)
    bv = nc.declare_dram_parameter("bv", [DG], F32, isOutput=False)
    bo = nc.declare_dram_parameter("bo", [D], F32, isOutput=False)
    expander_d = nc.declare_dram_parameter("expander", [128, 128], BF16, isOutput=False)
    # Two bf16 half-partials (pairs 0+1, pairs 2+3): the first is computed
    # during the last pair's otherwise-idle slots, the second at the tail.
    # The host sums both halves of both cores per batch (+bo) in f32.
    out = nc.declare_dram_parameter("out", [2, S, D], BF16, isOutput=True)

    Exp = mybir.ActivationFunctionType.Exp

    with tile.TileContext(nc) as tc:
        with ExitStack() as ctx:
            const = ctx.enter_context(tc.tile_pool(name="const", bufs=1))
            xpool = ctx.enter_context(tc.tile_pool(name="xres", bufs=1))
            wqk = ctx.enter_context(tc.tile_pool(name="wqk", bufs=2))
            wv_pool = ctx.enter_context(tc.tile_pool(name="wv", bufs=1))
            qk_pool = ctx.enter_context(tc.tile_pool(name="qk", bufs=2))
            vg_pool = ctx.enter_context(tc.tile_pool(name="vg", bufs=1))
            exp_pool = ctx.enter_context(tc.tile_pool(name="expp", bufs=18))
            small = ctx.enter_context(tc.tile_pool(name="small", bufs=2))
            # 8-deep: each evac copy can only reuse a slot after its DMA
            # completes (~2us round trip incl. the 900ns DMA-sem overhead),
            # so a shallow ring stalls the copy->matmul pipeline at the tail
            out_pool = ctx.enter_context(tc.tile_pool(name="outp", bufs=8))
            wo_pool = ctx.enter_context(tc.tile_pool(name="wo", bufs=2))

            spsum = ctx.enter_context(tc.tile_pool(name="sp", bufs=2, space="PSUM"))
            apsum = ctx.enter_context(tc.tile_pool(name="ap", bufs=1, space="PSUM"))
            qpsum = ctx.enter_context(tc.tile_pool(name="qp", bufs=2, space="PSUM"))
            drpool = ctx.enter_context(tc.tile_pool(name="dr", bufs=2, space="DRAM"))

            # Bias/expander tiles; DMAs are emitted after the x^T wave-1
            # dispatches (the DMA rings hold ~8 outstanding dispatches, and
            # pair-0's weights + wave-1 slices must own the ring heads).
            bq_sb = const.tile([128, NPAIR], F32)
            bk_sb = const.tile([128, NPAIR], F32)
            bv_bc = const.tile([128, DG], F32)
            # 0/1 block pattern: bc = expander.T @ [2,512] replicates moving
            # row 0 onto output partitions 0..63 and row 1 onto 64..127
            # (host-provided: engine memsets can't address partition base 1)
            expander = const.tile([128, 128], BF16)

            def load_consts(which):
                if which == 0:
                    nc.scalar.dma_start(out=bk_sb[:], in_=bk[:, :])
                    nc.scalar.dma_start(out=bq_sb[:], in_=bq[:, :])
                else:
                    nc.scalar.dma_start(
                        out=bv_bc[:], in_=_bcast_ap(bv, 128, 0, DG)
                    )
                    nc.scalar.dma_start(out=expander[:], in_=expander_d[:, :])

            # Residents: x^T [d, s] as 8 partition-tiles split across the
            # scalar + gpsimd DMA queues (the sync queue carries the pair-0
            # Wk/Wq and Wv loads first so projection starts immediately).
            xT_sb = xpool.tile([128, 8, S], BF16)
            xT_r = xT.rearrange("(k p) s -> p k s", p=128)

            def load_xt(s_lo, s_hi):
                dma_engines = [nc.scalar, nc.gpsimd]
                for k in range(8):
                    dma_engines[k % 2].dma_start(
                        out=xT_sb[:, k, s_lo:s_hi], in_=xT_r[:, k, s_lo:s_hi]
                    )

            attnT = xpool.tile([128, NPAIR, S], BF16)

            def make_v_steps():
                steps = []
                state = {}
                for skt in range(NSKT):
                    for k in range(8):
                        def step(skt=skt, k=k):
                            if k == 0:
                                state["ps"] = qpsum.tile(
                                    [128, 512], F32, tag="qp", name="vps"
                                )
                            ps = state["ps"]
                            nc.tensor.matmul(
                                ps[:],
                                xT_sb[:, k, 128 * skt : 128 * (skt + 1)],
                                wv_g[:, k, :],
                                start=(k == 0),
                                stop=(k == 7),
                            )
                            if k == 7:
                                nc.vector.tensor_add(
                                    vg[:, skt, :, 0:64],
                                    ps[:].rearrange("p (h d) -> p h d", h=8),
                                    bv_bc[:].rearrange("p (h d) -> p h d", h=8),
                                )
                        steps.append(step)
                return steps

            def start_pair_proj(p, dma_eng=None):
                """DMA the pair's Wq/Wk slices, allocate per-cq-chunk qt /
                zero-padded ktp tiles (chunked so scores only wait on the
                chunks they read), return the 64 per-k-step emission closures.
                Order [K0, Q0, K1, K2, K3, Q1, Q2, Q3]: slot (p, 0) scores
                sweep all K chunks but only Q chunk 0.  Pair 0 passes the
                gpsimd queue: the sync queue doesn't dispatch its first DMA
                until ~14us in, which stalled K0 ~5us."""
                if dma_eng is None:
                    dma_eng = nc.sync
                wk_p = wqk.tile([128, 8, 128], BF16, tag="wk")
                dma_eng.dma_start(out=wk_p[:], in_=Wk[p])
                wq_p = wqk.tile([128, 8, 128], BF16, tag="wq")
                dma_eng.dma_start(out=wq_p[:], in_=Wq[p])
                qt_cs = [
                    qk_pool.tile([128, 512], BF16, tag=f"qt{c}", name=f"qt{c}")
                    for c in range(NCQ)
                ]
                kt_cs = [
                    qk_pool.tile([128, 2, 512], BF16, tag=f"kt{c}", name=f"kt{c}")
                    for c in range(NCQ)
                ]
                def emit_memsets():
                    # gpsimd, not DVE: keeps the zero-pad fills off the DVE
                    # queue so the K-chunk bias adds (which gate the first
                    # scores of the pair) aren't stuck behind them
                    for c in range(NCQ):
                        nc.gpsimd.memset(kt_cs[c][64:128, 0, :], 0.0)
                        nc.gpsimd.memset(kt_cs[c][0:64, 1, :], 0.0)

                if dma_eng is nc.gpsimd:
                    # pair 0: defer so the memsets (gpsimd engine ops) don't
                    # sit between the weight DMAs and the x^T wave-1
                    # dispatches on the shared gpsimd sequencer stream
                    deferred_memsets.append(emit_memsets)
                else:
                    emit_memsets()
                state = {}

                def kstep(c, k):
                    if k == 0:
                        state["ps"] = qpsum.tile(
                            [128, 512], F32, tag="qp", name="kps"
                        )
                    ps = state["ps"]
                    nc.tensor.matmul(
                        ps[:],
                        wk_p[:, k, :],
                        xT_sb[:, k, 512 * c : 512 * (c + 1)],
                        start=(k == 0),
                        stop=(k == 7),
                    )
                    if k == 7:
                        nc.vector.tensor_scalar_add(
                            kt_cs[c][0:64, 0, :],
                            ps[0:64, :],
                            bk_sb[0:64, p : p + 1],
                        )
                        nc.vector.tensor_scalar_add(
                            kt_cs[c][64:128, 1, :],
                            ps[64:128, :],
                            bk_sb[64:128, p : p + 1],
                        )

                def qstep(c, k):
                    if k == 0:
                        state["ps"] = qpsum.tile(
                            [128, 512], F32, tag="qp", name="qps"
                        )
                    ps = state["ps"]
                    nc.tensor.matmul(
                        ps[:],
                        wq_p[:, k, :],
                        xT_sb[:, k, 512 * c : 512 * (c + 1)],
                        start=(k == 0),
                        stop=(k == 7),
                    )
                    if k == 7:
                        nc.vector.tensor_scalar_add(
                            qt_cs[c][:, :], ps[:], bq_sb[:, p : p + 1]
                        )

                chunk_order = [("k", 0), ("q", 0), ("k", 1), ("k", 2), ("k", 3),
                               ("q", 1), ("q", 2), ("q", 3)]
                steps = []
                for which, c in chunk_order:
                    fn = kstep if which == "k" else qstep
                    for k in range(8):
                        steps.append(
                            (lambda fn=fn, c=c, k=k: fn(c, k))
                        )
                return qt_cs, kt_cs, steps

            ex_ring = {}
            pair_states = {}
            deferred_memsets = []

            def make_attn_steps(p, cq):
                """17 closures: 16 lagged attn matmul pairs + psum evacuation."""
                steps = []
                state = {}
                for skt in range(NSKT):
                    def step(skt=skt):
                        if skt == 0:
                            state["aA"] = apsum.tile([65, 512], F32, tag="aA", name="aA")
                            state["aB"] = apsum.tile([65, 512], F32, tag="aB", name="aB")
                        exs = ex_ring.pop((p, cq, skt))
                        nc.tensor.matmul(
                            state["aA"][:],
                            vg[:, skt, 2 * p, :],
                            exs[:, 0:512],
                            start=(skt == 0),
                            stop=(skt == NSKT - 1),
                        )
                        nc.tensor.matmul(
                            state["aB"][:],
                            vg[:, skt, 2 * p + 1, :],
                            exs[:, 512:1024],
                            start=(skt == 0),
                            stop=(skt == NSKT - 1),
                        )
                    steps.append(step)

                def evac():
                    st = pair_states.setdefault(p, {})
                    if "sums" not in st:
                        st["sums"] = small.tile(
                            [128, 1024], F32, tag="sums", name="sums"
                        )
                        # fill with 1.0: the batched reciprocal covers all
                        # 128 partitions, and 1/garbage on the 124 unused
                        # rows can be inf/nan, which the broadcast matmul's
                        # 0-weight would turn into NaN (0*inf)
                        nc.gpsimd.memset(st["sums"][:], 1.0)
                    sums = st["sums"]
                    # both denominator rows first: the normalize reciprocals
                    # (next on the DVE stream at slot boundaries and at the
                    # tail) only need sums, so they start ~1.3us earlier
                    for half, key in ((0, "aA"), (1, "aB")):
                        at = state[key]
                        # chunk (cq, half) parks at 32-aligned partition
                        # 32*(2*(cq%2)+half), column block 512*(cq//2)
                        nc.vector.tensor_copy(
                            sums[
                                32 * (2 * (cq % 2) + half) : 32
                                * (2 * (cq % 2) + half)
                                + 1,
                                512 * (cq // 2) : 512 * (cq // 2 + 1),
                            ],
                            at[64:65, :],
                        )
                    for half, key in ((0, "aA"), (1, "aB")):
                        at = state[key]
                        nc.vector.tensor_copy(
                            attnT[
                                64 * half : 64 * (half + 1),
                                p,
                                512 * cq : 512 * (cq + 1),
                            ],
                            at[0:64, :],
                        )
                steps.append(evac)
                return steps

            def make_norm_steps(p, g):
                """Normalize pair p's cq chunks {2g, 2g+1}: a DVE reciprocal
                (split into 4 column pieces so no single DVE op blocks the
                queue longer than ~1us -- the proj-chunk bias adds that gate
                the PE's qp psum rotation share that queue), then per cq a PE
                broadcast matmul (expander replicates reciprocal rows 0/32
                across partitions 0-63/64-127) and one full-width DVE
                multiply.  Returned as closures so the slot loop can emit
                them interleaved with the skt stream."""
                state = {}

                def recip_piece(i):
                    if "rr" not in state:
                        state["rr"] = small.tile(
                            [128, 512], BF16, tag="rr", name="rr"
                        )
                    sums = pair_states[p]["sums"]
                    # bf16 reciprocal output: keeps the broadcast matmul in
                    # 1-cycle/row bf16 mode (fp32 matmuls are 4 cyc/row);
                    # ~0.1% rounding on the denominator is well in budget
                    with nc.allow_low_precision("bf16 softmax denominators"):
                        nc.vector.reciprocal(
                            state["rr"][:, 128 * i : 128 * (i + 1)],
                            sums[
                                :, 512 * g + 128 * i : 512 * g + 128 * (i + 1)
                            ],
                        )

                def bc_mul(cq):
                    rr = state["rr"]
                    bc = qpsum.tile([128, 512], F32, tag="qp", name="bc")
                    nc.tensor.matmul(
                        bc[:],
                        expander[64 * (cq % 2) : 64 * (cq % 2) + 64, :],
                        rr[64 * (cq % 2) : 64 * (cq % 2) + 64, :],
                        start=True,
                        stop=True,
                    )
                    sl = attnT[:, p, 512 * cq : 512 * (cq + 1)]
                    nc.vector.tensor_mul(sl, sl, bc[:])

                return [lambda i=i: recip_piece(i) for i in range(4)] + [
                    lambda cq=cq: bc_mul(cq) for cq in (2 * g, 2 * g + 1)
                ]

            def normalize_half(p, g):
                for s in make_norm_steps(p, g):
                    s()

            # --- prologue: pair-0 Wk/Wq lead the sync queue (then Wv), x^T
            # on the scalar/gpsimd queues in two waves (seq 0:512 first, so
            # K0/Q0 and scores can start ~40us earlier than a full-x^T
            # wait).  Only K0+Q0 run before slot 0; K1..Q3 and ALL V-chunk
            # projection steps trickle through slot 0 as extras, keeping the
            # PE saturated while the exp stream starts immediately. ---
            # Pair-0 weights lead the gpsimd queue, then x^T wave 1 on the
            # scalar/gpsimd queues; pair-0's kt memsets (gpsimd engine) come
            # after those dispatches so they don't block the queue.
            qt_cur, kt_cur, p0_steps = start_pair_proj(0, dma_eng=nc.gpsimd)
            load_xt(0, 512)
            load_consts(0)
            # seq 512:1024 ahead of the bulky const DMAs: slot-0's V-chunk
            # extras (chunks 4-7) read it by ~skt 6 and stall the PE if the
            # bv broadcast/expander sit ahead of it on the scalar queue
            load_xt(512, 1024)
            load_consts(1)
            for m in deferred_memsets:
                m()
            wv_g = wv_pool.tile([128, 8, DG], BF16)
            nc.sync.dma_start(out=wv_g[:], in_=Wv[:, :, :])
            vg = vg_pool.tile([128, NSKT, 8, 65], BF16)
            # ones column (index 64) via DVE memset: keeps the first EXP's
            # ACT stream free of any DMA-gated instruction (an ACT-side init
            # would serialize table-load + init + first-exp behind the bv
            # broadcast and stall the scores psum rotation ~6us)
            nc.vector.memset(vg[:, :, :, 64:65], 1.0)
            # dummy activation: pulls the 1.28us exp table load to the very
            # start instead of in front of the first real EXP
            scratch = const.tile([1, 4], F32)
            nc.scalar.activation(
                scratch[0:1, 0:4],
                vg[0:1, 0, 0:4, 64],
                Exp,
            )
            # PE warm-up: ~20 junk matmuls on whatever attnT holds, issued
            # while x^T streams in.  Sustained PE activity flips the HAM
            # clock gate to 8/8 (~3.4us of busy-ness) so the real prologue
            # matmuls run at 2.4GHz instead of 1.2 (saves ~15us of cold-rate
            # prologue; results land in a scratch psum tile, never read).
            for _ in range(20):
                jp = qpsum.tile([128, 512], F32, tag="qp", name="jp")
                nc.tensor.matmul(
                    jp[:], attnT[:, 0, 0:128], attnT[:, 0, 1024:1536],
                    start=True, stop=True,
                )
            # K0 + Q0 only (16 steps); the rest rides slot 0's extras.
            for s in p0_steps[:16]:
                s()
            load_xt(1024, S)
            v_steps = make_v_steps()
            v_steps = p0_steps[16:] + v_steps

            # --- main slot stream ---
            slots = [(p, cq) for p in range(NPAIR) for cq in range(NCQ)]
            attn_prev = None
            qt_next = kt_next = None
            proj_next = []

            wo_tiles = []
            norm_pending = []
            OC = 512

            def make_out_steps(half):
                """Half-output-projection steps: partial over pairs (2h, 2h+1)
                into out[half] as bf16.  half 0 trickles through the last
                pair's slots (its attnT is normalized by then and the PE has
                no proj work left); half 1 runs at the tail, ordered so the
                already-normalized q-chunks (t<8, i.e. cq 0/1) run while the
                final normalize chain (reciprocal+bounce+mul) completes."""
                steps = []
                order = [(c, t) for t in range(S // 128) for c in range(D // OC)]
                if half == 1:
                    order = [ct for ct in order if ct[1] < 8] + [
                        ct for ct in order if ct[1] >= 8
                    ]
                for si_, (c, t) in enumerate(order):
                    def step(c=c, t=t, half=half, si_=si_):
                        # tail: deepen the psum rotation to 4 by borrowing
                        # the scores pool's slots (idle once the last exp is
                        # done); reuse the existing tags so no extra PSUM is
                        # allocated
                        if half == 1 and si_ % 2:
                            ps = spsum.tile([128, OC], F32, tag="sc", name="op")
                        else:
                            ps = qpsum.tile([128, OC], F32, tag="qp", name="op")
                        for i in range(2):
                            k = 2 * half + i
                            nc.tensor.matmul(
                                ps[:],
                                attnT[:, k, 128 * t : 128 * (t + 1)],
                                wo_tiles[c][:, k, :],
                                start=(i == 0),
                                stop=(i == 1),
                            )
                        ot = out_pool.tile([128, OC], BF16, tag="ot")
                        # half 0 runs while ACT still paces exp: DVE only.
                        # tail: first 8 steps on ACT (DVE owns the normalize
                        # chain then), after that alternate ACT/DVE.
                        if half == 1 and (si_ < 8 or si_ % 2 == 0):
                            nc.scalar.copy(ot[:], ps[:])
                        else:
                            nc.vector.tensor_copy(ot[:], ps[:])
                        dma_eng = nc.sync if si_ % 2 == 0 else nc.scalar
                        dma_eng.dma_start(
                            out=out[
                                half,
                                128 * t : 128 * (t + 1),
                                OC * c : OC * (c + 1),
                            ],
                            in_=ot[:],
                        )
                    steps.append(step)
                return steps
            for si, (p, cq) in enumerate(slots):
                if cq == 0 and p + 1 < NPAIR:
                    # kick off next pair's weight DMAs + step list one full
                    # pair ahead of first use
                    qt_next, kt_next, proj_next = start_pair_proj(p + 1)
                if si == 11:
                    # prefetch the output projection's Wo chunks
                    for c in range(2):
                        wo_c = wo_pool.tile([128, NPAIR, 512], BF16, tag="wo")
                        nc.sync.dma_start(out=wo_c[:], in_=Wo[c])
                        wo_tiles.append(wo_c)
                if p == NPAIR - 1 and cq == 1:
                    # pairs 0+1 are normalized; their half of the output
                    # projection fills this pair's otherwise-idle PE slots
                    proj_next = make_out_steps(0)
                if si == 0:
                    extras = v_steps
                    v_steps = []
                else:
                    # spread next-pair proj over this pair's remaining slots
                    rem_slots = NCQ - cq
                    take = (
                        (len(proj_next) + rem_slots - 1) // rem_slots
                        if proj_next
                        else 0
                    )
                    extras = proj_next[:take]
                    proj_next = proj_next[take:]
                quota = (len(extras) + NSKT - 1) // NSKT if extras else 0

                ei = 0
                for skt in range(NSKT):
                    if norm_pending and skt % 2 == 0 and skt >= 2:
                        norm_pending.pop(0)()
                    if skt == NSKT - 1 and attn_prev is not None:
                        # early evacuation: previous slot's attn finished at
                        # skt==14 (double-step), so its psum drains during
                        # this iteration and the next slot starts stall-free
                        attn_prev[16]()
                    sc = spsum.tile([128, 1024], F32, tag="sc")
                    ktc = kt_cur[skt // 4]
                    lo = 128 * (skt % 4)
                    nc.tensor.matmul(
                        sc[:, 0:512],
                        ktc[:, 0, lo : lo + 128],
                        qt_cur[cq][:, :],
                        start=True,
                        stop=True,
                    )
                    nc.tensor.matmul(
                        sc[:, 512:1024],
                        ktc[:, 1, lo : lo + 128],
                        qt_cur[cq][:, :],
                        start=True,
                        stop=True,
                    )
                    ex = exp_pool.tile([128, 1024], BF16, tag="ex")
                    nc.scalar.activation(ex[:], sc[:], Exp, scale=0.125)
                    ex_ring[(p, cq, skt)] = ex

                    for _ in range(quota):
                        if ei < len(extras):
                            extras[ei]()
                            ei += 1
                    if attn_prev is not None:
                        if skt < NSKT - 2:
                            attn_prev[skt]()
                        elif skt == NSKT - 2:
                            attn_prev[NSKT - 2]()
                            attn_prev[NSKT - 1]()
                while ei < len(extras):
                    extras[ei]()
                    ei += 1
                while norm_pending:
                    norm_pending.pop(0)()
                if attn_prev is not None:
                    if cq == 0 and p > 0:
                        norm_pending = make_norm_steps(p - 1, 1)
                    elif cq == 2:
                        norm_pending = make_norm_steps(p, 0)
                attn_prev = make_attn_steps(p, cq)
                if cq == NCQ - 1 and p + 1 < NPAIR:
                    qt_cur, kt_cur = qt_next, kt_next

            # --- tail: last slot's attention, then the pairs-2+3 half of
            # the output projection.  The t<8 steps only touch cq 0/1
            # (already normalized), so they keep the PE busy while the last
            # normalize chain (evac -> reciprocal -> bounce -> mul) runs on
            # DVE; the t>=8 steps follow it. ---
            outb = make_out_steps(1)
            for skt in range(NSKT):
                attn_prev[skt]()
            attn_prev[16]()
            for s in outb[:6]:
                s()
            normalize_half(NPAIR - 1, 1)
            for s in outb[6:]:
                s()

    _split_all_multiwaits(nc)
    return nc


_NC_CACHE = None


def _get_nc():
    global _NC_CACHE
    if _NC_CACHE is None:
        _NC_CACHE = build_kernel()
    return _NC_CACHE


def make_in_maps(x, Wq, bq, Wk, bk, Wv, bv, Wo, bo):
    import ml_dtypes

    bf16 = ml_dtypes.bfloat16
    x = np.asarray(x, dtype=np.float32)
    Wq = np.asarray(Wq, dtype=np.float32)
    Wk = np.asarray(Wk, dtype=np.float32)
    Wv = np.asarray(Wv, dtype=np.float32)
    Wo = np.asarray(Wo, dtype=np.float32)
    bq = np.asarray(bq, dtype=np.float32)
    bk = np.asarray(bk, dtype=np.float32)
    bv = np.asarray(bv, dtype=np.float32)
    bo = np.asarray(bo, dtype=np.float32)
    bo_zero = np.zeros_like(bo)

    def qk_layout(w):
        # [D, DG] -> [pair, 128, k, 128]: d = 128k + p_row, col = 128*pair + cc
        return np.ascontiguousarray(
            w.reshape(8, 128, NPAIR, 128).transpose(2, 1, 0, 3).astype(bf16)
        )

    def v_layout(w):
        # [D, DG] -> [128, k, DG]
        return np.ascontiguousarray(
            w.reshape(8, 128, DG).transpose(1, 0, 2).astype(bf16)
        )

    def o_layout(w):
        # [DG, D] -> [chunk, 128, k, 512]: row = 128k + p_row, col = 512c + cc
        return np.ascontiguousarray(
            w.reshape(NPAIR, 128, 2, 512).transpose(2, 1, 0, 3).astype(bf16)
        )

    def b_layout(v):
        # [DG] -> [128, NPAIR]: elem 128k + p_row -> [p_row, k]
        return np.ascontiguousarray(v.reshape(NPAIR, 128).T)

    expander_host = np.zeros((128, 128), dtype=bf16)
    for blk in (0, 64):
        expander_host[blk + 0, 0:64] = 1
        expander_host[blk + 32, 64:128] = 1
    in_maps = []
    for c in range(NCORES):
        b, g = divmod(c, 2)
        lo, hi = DG * g, DG * (g + 1)
        xt = np.ascontiguousarray(x[b].T.astype(bf16))  # [D, S]
        in_maps.append(
            {
                "xT": xt,
                "expander": expander_host,
                "Wq": qk_layout(Wq[:, lo:hi]),
                "Wk": qk_layout(Wk[:, lo:hi]),
                "Wv": v_layout(Wv[:, lo:hi]),
                "Wo": o_layout(Wo[lo:hi, :]),
                "bq": b_layout(bq[lo:hi]),
                "bk": b_layout(bk[lo:hi]),
                "bv": np.ascontiguousarray(bv[lo:hi]),
                "bo": bo if g == 0 else bo_zero,
            }
        )
    return in_maps


def run(x, Wq, bq, Wk, bk, Wv, bv, Wo, bo, trace=False):
    nc = _get_nc()
    in_maps = make_in_maps(x, Wq, bq, Wk, bk, Wv, bv, Wo, bo)
    res = run_bass_kernel_spmd(
        nc, in_maps, core_ids=list(range(NCORES)), trace=trace
    )
    B = 4
    bo = np.asarray(bo, dtype=np.float32)
    out = np.empty((B, S, D), dtype=np.float32)
    for b in range(B):
        # the Wo row-split all-reduce + bias, host-side during unshard;
        # each core ships two bf16 half-partials (pairs 0+1, pairs 2+3)
        pa = np.asarray(res.results[2 * b]["out"], dtype=np.float32)
        pb = np.asarray(res.results[2 * b + 1]["out"], dtype=np.float32)
        out[b] = pa[0] + pa[1] + pb[0] + pb[1]
        out[b] += bo
    return out, res


def kernel(**inputs):
    out, _ = run(**inputs)
    return out

